# revision 5
# baseline (speedup 1.0000x reference)
"""Trainium2 Bass kernel for Tacotron-style attention (nn_Attention_12704513261859).

Computation (per batch b):
    e[t, h]   = tanh( cbhg[b] @ W1[:D] + rnn[b] @ W1[D:] + b1 )[t, h]
    en[t]     = relu( e[t, :] @ W2 + b2 )
    aw        = softmax(en over t)
    context   = aw @ cbhg[b]            -> [1, D]
    out2      = rnn reshaped [B, 1, D]  (pure reshape, done host-side)

Sharding: pure data-parallel over batch B=32 across 8 NeuronCores
(4 batches per core); tiny Dense weights replicated.

Kernel structure per core (4 batches):
  - DMA cbhg natural tiles [128(t), 512(d)] (8 per batch)
  - PE-transpose them into [128(d), t] tiles (the e-matmul contracts d,
    which must sit on partitions)
  - e-matmul:  psum[10, 512] += W1a_chunk[128,10].T @ xT_chunk[128,512]
  - tanh via ScalarE with per-partition bias = rnn@W1b + b1 (fused psum evac)
  - energies: per t-chunk matmul lhsT=e[:,128-slice] (K=10) rhs=W2 -> [128,1]
  - exp(relu(x+b2)) == max(exp(x+b2), 1): ACT Exp + DVE max (exact identity)
  - unnormalized context: psum[1,512] += expE_chunk[128,1].T @ x_tile[128,512]
  - denominator via ones-matmul + DVE reduce/reciprocal; final scale on ACT
"""

import os
import numpy as np

B, T, D, H = 32, 1024, 512, 10
NCORES = 8
BP = B // NCORES          # batches per core = 4
NT = T // 128             # 8 t-chunks
ND = D // 128             # 4 d-chunks

_CACHE = {}


def _build_nc():
    from contextlib import ExitStack

    import concourse.bass as bass
    import concourse.mybir as mybir
    import concourse.tile as tile
    from concourse import bacc
    from concourse.masks import make_identity

    f32 = mybir.dt.float32

    nc = bacc.Bacc("TRN2", target_bir_lowering=False, debug=False, num_devices=NCORES)

    x_d = nc.dram_tensor("x", [BP, T, D], f32, kind="ExternalInput")
    rnn_d = nc.dram_tensor("rnn", [BP, D], f32, kind="ExternalInput")
    w1_d = nc.dram_tensor("w1", [2 * D, H], f32, kind="ExternalInput")
    b1_d = nc.dram_tensor("b1", [H], f32, kind="ExternalInput")
    w2_d = nc.dram_tensor("w2", [H, 1], f32, kind="ExternalInput")
    b2_d = nc.dram_tensor("b2", [1], f32, kind="ExternalInput")
    out_d = nc.dram_tensor("ctx_out", [BP, D], f32, kind="ExternalOutput")

    with tile.TileContext(nc) as tc, ExitStack() as ctx:
        consts = ctx.enter_context(tc.tile_pool(name="consts", bufs=1))
        xp = ctx.enter_context(tc.tile_pool(name="xp", bufs=BP))
        xtp = ctx.enter_context(tc.tile_pool(name="xtp", bufs=BP))
        ep = ctx.enter_context(tc.tile_pool(name="ep", bufs=2))
        sp = ctx.enter_context(tc.tile_pool(name="sp", bufs=2))
        op = ctx.enter_context(tc.tile_pool(name="op", bufs=2))
        ptr = ctx.enter_context(tc.tile_pool(name="ptr", bufs=2, space="PSUM"))
        pe_ps = ctx.enter_context(tc.tile_pool(name="pe_ps", bufs=2, space="PSUM"))
        psm = ctx.enter_context(tc.tile_pool(name="psm", bufs=2, space="PSUM"))
        pcx = ctx.enter_context(tc.tile_pool(name="pcx", bufs=2, space="PSUM"))

        # ---- constants / weights ----
        ident = consts.tile([128, 128], f32)
        make_identity(nc, ident)
        ones = consts.tile([128, 1], f32)
        nc.vector.memset(ones, 1.0)

        w1a = consts.tile([128, ND, H], f32)
        nc.gpsimd.dma_start(
            out=w1a, in_=w1_d[0:D, :].rearrange("(c p) h -> p c h", p=128)
        )
        w1b = consts.tile([128, ND, H], f32)
        nc.gpsimd.dma_start(
            out=w1b, in_=w1_d[D : 2 * D, :].rearrange("(c p) h -> p c h", p=128)
        )
        w2s = consts.tile([H, 1], f32)
        nc.gpsimd.dma_start(out=w2s, in_=w2_d[:, :])
        b1s = consts.tile([H, 1], f32)
        nc.gpsimd.dma_start(out=b1s, in_=b1_d[:].rearrange("(h o) -> h o", o=1))
        b2s = consts.tile([128, 1], f32)
        b2_ap = b2_d[:]
        b2_bcast = bass.AP(
            tensor=b2_ap.tensor, offset=b2_ap.offset, ap=[[0, 128]] + list(b2_ap.ap)
        )
        nc.gpsimd.dma_start(out=b2s, in_=b2_bcast)
        rnnT = consts.tile([128, ND, BP], f32)
        for c in range(ND):
            nc.gpsimd.dma_start(
                out=rnnT[:, c, :],
                in_=rnn_d[:, c * 128 : (c + 1) * 128].rearrange("b p -> p b"),
            )

        # ---- r = rnn @ W1b + b1 for all local batches: [10, BP] ----
        rps = psm.tile([H, BP], f32, tag="small", name="rps")
        for c in range(ND):
            nc.tensor.matmul(
                rps, w1b[:, c, :], rnnT[:, c, :], start=(c == 0), stop=(c == ND - 1)
            )
        rb = consts.tile([H, BP], f32)
        nc.vector.tensor_scalar_add(rb, rps, b1s[:, 0:1])

        # ---- bulk x loads (natural layout: t on partitions) ----
        x_tiles = []
        for b in range(BP):
            xb = xp.tile([128, NT, D], f32, tag="xb", name=f"xb{b}")
            src = x_d[b].rearrange("(i p) d -> p i d", p=128)
            nc.sync.dma_start(out=xb[:, 0:4, :], in_=src[:, 0:4, :])
            nc.sync.dma_start(out=xb[:, 4:8, :], in_=src[:, 4:8, :])
            x_tiles.append(xb)

        xt_tiles = [None] * BP
        e_tiles = [None] * BP

        def transposes(b):
            xt = xtp.tile([128, ND, T], f32, tag="xt", name=f"xt{b}")
            xt_tiles[b] = xt
            k = 0
            for j in range(ND):
                for half in range(2):
                    tps = ptr.tile([128, 512], f32, tag="tps", name=f"tps{b}_{j}_{half}")
                    for q in range(4):
                        i = half * 4 + q
                        nc.tensor.transpose(
                            tps[:, q * 128 : (q + 1) * 128],
                            x_tiles[b][:, i, j * 128 : (j + 1) * 128],
                            ident,
                        )
                    dst = xt[:, j, half * 512 : (half + 1) * 512]
                    if k % 2 == 0:
                        nc.vector.tensor_copy(dst, tps)
                    else:
                        nc.scalar.copy(dst, tps)
                    k += 1

        def e_mm(b):
            e_sb = ep.tile([H, T], f32, tag="e", name=f"e{b}")
            e_tiles[b] = e_sb
            for n in range(2):
                eps = pe_ps.tile([H, 512], f32, tag="eps", name=f"eps{b}_{n}")
                for j in range(ND):
                    nc.tensor.matmul(
                        eps,
                        w1a[:, j, :],
                        xt_tiles[b][:, j, n * 512 : (n + 1) * 512],
                        start=(j == 0),
                        stop=(j == ND - 1),
                    )
                nc.scalar.activation(
                    e_sb[:, n * 512 : (n + 1) * 512],
                    eps,
                    mybir.ActivationFunctionType.Tanh,
                    bias=rb[:, b : b + 1],
                    scale=1.0,
                )

        def tail(b):
            # energies per t-chunk, t on partitions: [128, NT]
            enps = psm.tile([128, NT], f32, tag="small", name=f"en{b}")
            for i in range(NT):
                nc.tensor.matmul(
                    enps[:, i : i + 1],
                    e_tiles[b][:, i * 128 : (i + 1) * 128],
                    w2s,
                    start=True,
                    stop=True,
                )
            # exp(relu(x + b2)) == max(exp(x + b2), 1)
            expE = sp.tile([128, NT], f32, tag="expE", name=f"expE{b}")
            nc.scalar.activation(
                expE, enps, mybir.ActivationFunctionType.Exp, bias=b2s[:, 0:1], scale=1.0
            )
            nc.vector.tensor_scalar_max(expE, expE, 1.0)
            # denominator = sum_t expE
            dps = psm.tile([1, NT], f32, tag="small", name=f"dps{b}")
            nc.tensor.matmul(dps, ones, expE, start=True, stop=True)
            den = sp.tile([1, 2], f32, tag="den", name=f"den{b}")
            nc.vector.reduce_sum(out=den[:, 0:1], in_=dps, axis=mybir.AxisListType.X)
            nc.vector.reciprocal(den[:, 1:2], den[:, 0:1])
            # unnormalized context
            cps = pcx.tile([1, D], f32, tag="cps", name=f"cps{b}")
            for i in range(NT):
                nc.tensor.matmul(
                    cps,
                    expE[:, i : i + 1],
                    x_tiles[b][:, i, :],
                    start=(i == 0),
                    stop=(i == NT - 1),
                )
            ctx_sb = op.tile([1, D], f32, tag="ctx", name=f"ctx{b}")
            nc.scalar.mul(ctx_sb, cps, den[:, 1:2])
            nc.sync.dma_start(out=out_d[b : b + 1, :], in_=ctx_sb)

        # software-pipelined emission: batch b+1 transposes keep the PE busy
        # while batch b's activations run on ScalarE/VectorE
        transposes(0)
        e_mm(0)
        for b in range(BP):
            if b + 1 < BP:
                transposes(b + 1)
            tail(b)
            if b + 1 < BP:
                e_mm(b + 1)

    nc.compile()
    return nc


def _get_nc():
    if "nc" not in _CACHE:
        _CACHE["nc"] = _build_nc()
    return _CACHE["nc"]


def _make_in_maps(cbhg, rnn, w1, b1, w2, b2):
    return [
        {
            "x": np.ascontiguousarray(cbhg[c * BP : (c + 1) * BP]),
            "rnn": np.ascontiguousarray(rnn[c * BP : (c + 1) * BP]),
            "w1": w1,
            "b1": b1,
            "w2": w2,
            "b2": b2,
        }
        for c in range(NCORES)
    ]


def _run(in_maps, trace=False):
    from concourse.bass_utils import run_bass_kernel_spmd

    nc = _get_nc()
    return run_bass_kernel_spmd(nc, in_maps, core_ids=list(range(NCORES)), trace=trace)


def kernel(cbhg_encoding, attention_rnn_output, W1, b1, W2, b2):
    cbhg = np.asarray(cbhg_encoding, dtype=np.float32)
    rnn = np.asarray(attention_rnn_output, dtype=np.float32)
    w1 = np.ascontiguousarray(np.asarray(W1, dtype=np.float32))
    b1v = np.ascontiguousarray(np.asarray(b1, dtype=np.float32))
    w2 = np.ascontiguousarray(np.asarray(W2, dtype=np.float32))
    b2v = np.ascontiguousarray(np.asarray(b2, dtype=np.float32))

    res = _run(_make_in_maps(cbhg, rnn, w1, b1v, w2, b2v))
    context = np.concatenate(
        [res.results[c]["ctx_out"][:, None, :] for c in range(NCORES)], axis=0
    ).astype(np.float32)
    rnn_reshaped = rnn.reshape(B, 1, D).copy()
    return (context, rnn_reshaped)


# revision 6
# speedup vs baseline: 1.7813x; 1.7813x over previous
"""Trainium2 Bass kernel for Tacotron-style attention (nn_Attention_12704513261859).

Computation (per batch b):
    e[t, h]   = tanh( cbhg[b] @ W1[:D] + rnn[b] @ W1[D:] + b1 )[t, h]
    en[t]     = relu( e[t, :] @ W2 + b2 )
    aw        = softmax(en over t)
    context   = aw @ cbhg[b]            -> [1, D]
    out2      = rnn reshaped [B, 1, D]  (pure reshape, done host-side)

Sharding: pure data-parallel over batch B=32 across 8 NeuronCores
(4 batches per core); tiny Dense weights replicated.

fp32 matmuls on the TRN2 PE cost ~4x bf16 (hi/lo double pass at half
stream rate), so the matmul datapaths run in bf16 with fp32 PSUM
accumulation:
  - cast X tiles to bf16 once (ScalarE/VectorE split)
  - PE-transpose the bf16 tiles (FWL weight load) for the e-matmul,
    which must contract d on partitions
  - e-matmul / energies / context all bf16 inputs, fp32 accumulate
  - softmax denominator matches the bf16-rounded numerator weights
exp(relu(x + b2)) is computed exactly as max(exp(x + b2), 1).
"""

import os
import numpy as np

B, T, D, H = 32, 1024, 512, 10
NCORES = 8
BP = B // NCORES          # batches per core = 4
NT = T // 128             # 8 t-chunks
ND = D // 128             # 4 d-chunks

_CACHE = {}


def _build_nc():
    from contextlib import ExitStack

    import concourse.bass as bass
    import concourse.mybir as mybir
    import concourse.tile as tile
    from concourse import bacc
    from concourse.masks import make_identity

    f32 = mybir.dt.float32
    bf16 = mybir.dt.bfloat16

    nc = bacc.Bacc("TRN2", target_bir_lowering=False, debug=False, num_devices=NCORES)

    x_d = nc.dram_tensor("x", [BP, T, D], f32, kind="ExternalInput")
    rnn_d = nc.dram_tensor("rnn", [BP, D], f32, kind="ExternalInput")
    w1_d = nc.dram_tensor("w1", [2 * D, H], f32, kind="ExternalInput")
    b1_d = nc.dram_tensor("b1", [H], f32, kind="ExternalInput")
    w2_d = nc.dram_tensor("w2", [H, 1], f32, kind="ExternalInput")
    b2_d = nc.dram_tensor("b2", [1], f32, kind="ExternalInput")
    out_d = nc.dram_tensor("ctx_out", [BP, D], f32, kind="ExternalOutput")

    with tile.TileContext(nc) as tc, ExitStack() as ctx:
        consts = ctx.enter_context(tc.tile_pool(name="consts", bufs=1))
        xp = ctx.enter_context(tc.tile_pool(name="xp", bufs=BP))
        xbp = ctx.enter_context(tc.tile_pool(name="xbp", bufs=BP))
        xtp = ctx.enter_context(tc.tile_pool(name="xtp", bufs=BP))
        ep = ctx.enter_context(tc.tile_pool(name="ep", bufs=2))
        sp = ctx.enter_context(tc.tile_pool(name="sp", bufs=2))
        op = ctx.enter_context(tc.tile_pool(name="op", bufs=2))
        ptr = ctx.enter_context(tc.tile_pool(name="ptr", bufs=2, space="PSUM"))
        pe_ps = ctx.enter_context(tc.tile_pool(name="pe_ps", bufs=2, space="PSUM"))
        psm = ctx.enter_context(tc.tile_pool(name="psm", bufs=2, space="PSUM"))
        pcx = ctx.enter_context(tc.tile_pool(name="pcx", bufs=2, space="PSUM"))

        # ---- constants / weights ----
        ident = consts.tile([128, 128], bf16)
        make_identity(nc, ident)
        ones = consts.tile([128, 1], bf16)
        nc.vector.memset(ones, 1.0)

        w1a = consts.tile([128, ND, H], f32)
        nc.gpsimd.dma_start(
            out=w1a, in_=w1_d[0:D, :].rearrange("(c p) h -> p c h", p=128)
        )
        w1b = consts.tile([128, ND, H], f32)
        nc.gpsimd.dma_start(
            out=w1b, in_=w1_d[D : 2 * D, :].rearrange("(c p) h -> p c h", p=128)
        )
        w1a_b = consts.tile([128, ND, H], bf16)
        nc.vector.tensor_copy(w1a_b, w1a)
        w2s = consts.tile([H, 1], f32)
        nc.gpsimd.dma_start(out=w2s, in_=w2_d[:, :])
        w2s_b = consts.tile([H, 1], bf16)
        nc.vector.tensor_copy(w2s_b, w2s)
        b1s = consts.tile([H, 1], f32)
        nc.gpsimd.dma_start(out=b1s, in_=b1_d[:].rearrange("(h o) -> h o", o=1))
        b2s = consts.tile([128, 1], f32)
        b2_ap = b2_d[:]
        b2_bcast = bass.AP(
            tensor=b2_ap.tensor, offset=b2_ap.offset, ap=[[0, 128]] + list(b2_ap.ap)
        )
        nc.gpsimd.dma_start(out=b2s, in_=b2_bcast)
        rnnT = consts.tile([128, ND, BP], f32)
        for c in range(ND):
            nc.gpsimd.dma_start(
                out=rnnT[:, c, :],
                in_=rnn_d[:, c * 128 : (c + 1) * 128].rearrange("b p -> p b"),
            )

        # ---- r = rnn @ W1b + b1 for all local batches: [10, BP] (fp32) ----
        rps = psm.tile([H, BP], f32, tag="small", name="rps")
        for c in range(ND):
            nc.tensor.matmul(
                rps, w1b[:, c, :], rnnT[:, c, :], start=(c == 0), stop=(c == ND - 1)
            )
        rb = consts.tile([H, BP], f32)
        nc.vector.tensor_scalar_add(rb, rps, b1s[:, 0:1])

        # ---- bulk x loads (natural layout: t on partitions) ----
        x_tiles = []
        for b in range(BP):
            xb = xp.tile([128, NT, D], f32, tag="xb", name=f"xb{b}")
            src = x_d[b].rearrange("(i p) d -> p i d", p=128)
            nc.sync.dma_start(out=xb[:, 0:4, :], in_=src[:, 0:4, :])
            nc.sync.dma_start(out=xb[:, 4:8, :], in_=src[:, 4:8, :])
            x_tiles.append(xb)

        xc_tiles = [None] * BP   # bf16 casts of x
        xt_tiles = [None] * BP   # bf16 transposed
        e_tiles = [None] * BP

        def casts(b):
            xc = xbp.tile([128, NT, D], bf16, tag="xc", name=f"xc{b}")
            xc_tiles[b] = xc
            for i in range(NT):
                src = x_tiles[b][:, i, :]
                dst = xc[:, i, :]
                if i % 2 == 0:
                    nc.scalar.copy(dst, src)
                else:
                    nc.vector.tensor_copy(dst, src)

        def transposes(b):
            xt = xtp.tile([128, ND, T], bf16, tag="xt", name=f"xt{b}")
            xt_tiles[b] = xt
            k = 0
            for j in range(ND):
                for half in range(2):
                    tps = ptr.tile([128, 512], bf16, tag="tps", name=f"tps{b}_{j}_{half}")
                    for q in range(4):
                        i = half * 4 + q
                        nc.tensor.transpose(
                            tps[:, q * 128 : (q + 1) * 128],
                            xc_tiles[b][:, i, j * 128 : (j + 1) * 128],
                            ident,
                        )
                    dst = xt[:, j, half * 512 : (half + 1) * 512]
                    if k % 2 == 0:
                        nc.vector.tensor_copy(dst, tps)
                    else:
                        nc.scalar.copy(dst, tps)
                    k += 1

        def e_mm(b):
            e_sb = ep.tile([H, T], bf16, tag="e", name=f"e{b}")
            e_tiles[b] = e_sb
            for n in range(2):
                eps = pe_ps.tile([H, 512], f32, tag="eps", name=f"eps{b}_{n}")
                for j in range(ND):
                    nc.tensor.matmul(
                        eps,
                        w1a_b[:, j, :],
                        xt_tiles[b][:, j, n * 512 : (n + 1) * 512],
                        start=(j == 0),
                        stop=(j == ND - 1),
                    )
                nc.scalar.activation(
                    e_sb[:, n * 512 : (n + 1) * 512],
                    eps,
                    mybir.ActivationFunctionType.Tanh,
                    bias=rb[:, b : b + 1],
                    scale=1.0,
                )

        def tail(b):
            # energies per t-chunk, t on partitions: [128, NT] fp32 psum
            enps = psm.tile([128, NT], f32, tag="small", name=f"en{b}")
            for i in range(NT):
                nc.tensor.matmul(
                    enps[:, i : i + 1],
                    e_tiles[b][:, i * 128 : (i + 1) * 128],
                    w2s_b,
                    start=True,
                    stop=True,
                )
            # exp(relu(x + b2)) == max(exp(x + b2), 1); bf16 weights for the
            # context matmul, denominator computed from the SAME bf16 values
            exps = sp.tile([128, NT], f32, tag="exps", name=f"exps{b}")
            nc.scalar.activation(
                exps, enps, mybir.ActivationFunctionType.Exp, bias=b2s[:, 0:1], scale=1.0
            )
            expE = sp.tile([128, NT], bf16, tag="expE", name=f"expE{b}")
            nc.vector.tensor_scalar_max(expE, exps, 1.0)
            # denominator = sum_t expE
            dps = psm.tile([1, NT], f32, tag="small", name=f"dps{b}")
            nc.tensor.matmul(dps, ones, expE, start=True, stop=True)
            den = sp.tile([1, 2], f32, tag="den", name=f"den{b}")
            nc.vector.reduce_sum(out=den[:, 0:1], in_=dps, axis=mybir.AxisListType.X)
            nc.vector.reciprocal(den[:, 1:2], den[:, 0:1])
            # unnormalized context (bf16 inputs, fp32 accumulate)
            cps = pcx.tile([1, D], f32, tag="cps", name=f"cps{b}")
            for i in range(NT):
                nc.tensor.matmul(
                    cps,
                    expE[:, i : i + 1],
                    xc_tiles[b][:, i, :],
                    start=(i == 0),
                    stop=(i == NT - 1),
                )
            ctx_sb = op.tile([1, D], f32, tag="ctx", name=f"ctx{b}")
            nc.scalar.mul(ctx_sb, cps, den[:, 1:2])
            nc.sync.dma_start(out=out_d[b : b + 1, :], in_=ctx_sb)

        # software-pipelined emission: batch b+1 work keeps engines busy
        # while batch b's dependent tail runs
        casts(0)
        transposes(0)
        e_mm(0)
        for b in range(BP):
            if b + 1 < BP:
                casts(b + 1)
                transposes(b + 1)
            tail(b)
            if b + 1 < BP:
                e_mm(b + 1)

    nc.compile()
    return nc


def _get_nc():
    if "nc" not in _CACHE:
        _CACHE["nc"] = _build_nc()
    return _CACHE["nc"]


def _make_in_maps(cbhg, rnn, w1, b1, w2, b2):
    return [
        {
            "x": np.ascontiguousarray(cbhg[c * BP : (c + 1) * BP]),
            "rnn": np.ascontiguousarray(rnn[c * BP : (c + 1) * BP]),
            "w1": w1,
            "b1": b1,
            "w2": w2,
            "b2": b2,
        }
        for c in range(NCORES)
    ]


def _run(in_maps, trace=False):
    from concourse.bass_utils import run_bass_kernel_spmd

    nc = _get_nc()
    return run_bass_kernel_spmd(nc, in_maps, core_ids=list(range(NCORES)), trace=trace)


def kernel(cbhg_encoding, attention_rnn_output, W1, b1, W2, b2):
    cbhg = np.asarray(cbhg_encoding, dtype=np.float32)
    rnn = np.asarray(attention_rnn_output, dtype=np.float32)
    w1 = np.ascontiguousarray(np.asarray(W1, dtype=np.float32))
    b1v = np.ascontiguousarray(np.asarray(b1, dtype=np.float32))
    w2 = np.ascontiguousarray(np.asarray(W2, dtype=np.float32))
    b2v = np.ascontiguousarray(np.asarray(b2, dtype=np.float32))

    res = _run(_make_in_maps(cbhg, rnn, w1, b1v, w2, b2v))
    context = np.concatenate(
        [res.results[c]["ctx_out"][:, None, :] for c in range(NCORES)], axis=0
    ).astype(np.float32)
    rnn_reshaped = rnn.reshape(B, 1, D).copy()
    return (context, rnn_reshaped)
